# revision 28
# baseline (speedup 1.0000x reference)
"""Trainium2 Bass kernel for a fused GRUCell step.

Math (reference):
    xi = x @ [W_ir W_iz W_in] + [b_ir b_iz b_in]
    hh = h @ [W_hr W_hz W_hn]
    r = sigmoid(xr + hr); z = sigmoid(xz + hz)
    n = tanh(xn + r * (hn + b_hn))
    new_h = (1 - z) * n + z * h

Strategy: pure data-parallel over the batch dim (B=16384 -> 8 cores x 2048).
Weights replicated. Per core, one K-concatenated GEMM family with K = F + H
= 2048: stationary = batch block of xh^T, moving = per-gate weights.

Precision plan (the kernel is tensor-bound: fp16 PE floor is ~328us/core):
  - r and z gates run in fp8 e4m3 DoubleRow mode (0.5 cycles/row) — their
    quantization error is strongly attenuated through the sigmoid and the
    r*hn / z*(h-n) paths (measured ~1.9e-2 rel err on the full GRU).
  - n gate stays fp16 (it dominates output error; fp8 there fails 2e-2).
  - fp8 operands carry scales (acts x16, weights x512) to stay in e4m3's
    normal range; the descale folds into the sigmoid's scale argument.
  - h for the final blend and the output are fp16 (adds ~2e-4).

DMA plan: two passes over the m-tiles (output columns 0..511, then
512..1023) so the first pass needs only half the weights and the PE starts
~10us in; lhsT stays resident in SBUF across both passes. Bulk loads issue
on the sync (SP) DGE queue, dripped just-in-time through the loop — a
clump of large dma_starts fills the DGE ring and blocks the issuing
engine, which would stall compute if issued from scalar (measured: a 37us
PE stall). During the head only, pass-0's n-gate weights and first h
chunks ride the still-idle scalar queue so both sequencers generate
descriptors in parallel; mid-run the scalar queue only writes outputs.

Measured: 246us (baseline fp16 kernel: 401us); fp16 PE floor 328us, this
mix's floor 218us busy + ~8us preamble + ~7us tail.
"""

import os
import sys

import numpy as np

sys.path.insert(0, "/opt/trn_rl_repo")
os.environ.setdefault("MYCRO_LOCAL_CACHE", "1")

import ml_dtypes  # noqa: E402

import concourse.bass as bass  # noqa: E402
import concourse.mybir as mybir  # noqa: E402
import concourse.tile as tile  # noqa: E402
from concourse import bacc  # noqa: E402
from concourse.bass_utils import run_bass_kernel_spmd  # noqa: E402

N_CORES = 8
F = 1024  # input feature dim
H = 1024  # hidden dim
K = F + H  # GEMM contraction dim (x features then h features)
P = 128
KO = K // P  # 16 k-chunks of 128
KP = KO // 2  # 8 k-chunk pairs (DoubleRow processes 2 at a time)
NC_CHUNK = 512  # H columns per PSUM bank / matmul
HC_N = H // NC_CHUNK

ACT_SCALE = 16.0  # x,h ~ N(0,1) -> fp8 values ~N(0,16), well inside e4m3
W_SCALE = 512.0  # W ~ N(0,1/1024) -> fp8 values ~N(0,16)
INV_SCALE = 1.0 / (ACT_SCALE * W_SCALE)

# Per-gate precision for r and z: 'fp8' (both K-halves e4m3 DoubleRow),
# 'mixed' (x-half fp8, h-half fp16), or 'fp16'. n is always fp16.
R_MODE = "fp8"
Z_MODE = "fp8"


def build_gru_program(b_core: int, with_bias: bool, r_mode: str, z_mode: str) -> bass.Bass:
    """One SPMD program; every core runs it on its own batch shard.

    Two passes over the 16 m-tiles: pass hc=0 computes output columns
    0..511 (needs only half the weights, so the PE starts almost
    immediately and is never starved by the weight stream), pass hc=1 the
    rest. lhsT tiles stay resident in SBUF across both passes.
    """
    fp8 = mybir.dt.float8e4
    fp16 = mybir.dt.float16
    f32 = mybir.dt.float32
    n_m = b_core // P  # m-tiles of 128 batch rows
    assert b_core % P == 0
    any_fp8 = r_mode != "fp16" or z_mode != "fp16"
    DR = mybir.MatmulPerfMode.DoubleRow

    nc = bacc.Bacc()
    # Host-pretransposed layouts: partition dim first, contiguous per-line.
    lt16_d = nc.declare_dram_parameter("lt16", [n_m, P, KO, P], fp16, isOutput=False)
    if any_fp8:
        lt8_d = nc.declare_dram_parameter("lt8", [n_m, P, KO, P], fp8, isOutput=False)
    # r/z weights: [p, hc, kp, j, n] (j = the 2 k-subtiles of a DoubleRow pair)
    w8_d = {}
    w16_d = {}
    for g, mode in (("r", r_mode), ("z", z_mode)):
        if mode != "fp16":
            w8_d[g] = nc.declare_dram_parameter(
                f"w{g}8", [P, HC_N, KP, 2, NC_CHUNK], fp8, isOutput=False
            )
        if mode != "fp8":
            w16_d[g] = nc.declare_dram_parameter(
                f"w{g}16", [P, HC_N, KO, NC_CHUNK], fp16, isOutput=False
            )
    wn16_d = nc.declare_dram_parameter("wn16", [P, HC_N, KO, NC_CHUNK], fp16, isOutput=False)
    h16_d = nc.declare_dram_parameter("h16", [b_core, H], fp16, isOutput=False)
    if with_bias:
        # host-replicated across partitions; rows: b_r, b_z, b_in, b_hn
        bias_d = nc.declare_dram_parameter("bias_rep", [P, 4, H], f32, isOutput=False)
    out_d = nc.declare_dram_parameter("out", [b_core, H], fp16, isOutput=True)

    Sigmoid = mybir.ActivationFunctionType.Sigmoid
    Tanh = mybir.ActivationFunctionType.Tanh

    # Scale of each gate's psum relative to the true pre-activation.
    gate_scale = {
        "r": INV_SCALE if r_mode != "fp16" else 1.0,
        "z": INV_SCALE if z_mode != "fp16" else 1.0,
    }

    def weight_issuers(hc, wpool, w8sb, w16sb):
        """Thunks that each DMA one weight tile of pass `hc` (on sync: the
        SP queue has no compute, so a deep DGE ring is harmless). One tile
        per matmul operand: feeding matmuls from slices of a larger tile
        measured 20% slower per matmul (259ns vs 216ns — the sliced moving
        AP appears to defeat the LDWEIGHTS/stream overlap).

        Descriptor generation costs ~600ns per dma_start and paces the
        head: pass-0's n-gate weights go on the scalar queue (idle until
        the first sigmoid ~20us in) so both sequencers generate
        descriptors in parallel during the head burst."""
        wn_eng = nc.scalar if hc == 0 else nc.sync

        def wn_quad(q):
            # 512KB flat tile, one dma, 4 matmul operands as last-dim
            # slices. Unlike middle-dim-indexed slices (the 20% regression),
            # a contiguous 2D slice feeds the PE at full rate.
            t = wpool.tile([P, 4 * NC_CHUNK], fp16, tag=f"w16n{hc}q{q}", name="wn16sb")
            wn_eng.dma_start(t[:], wn16_d[:, hc, 4 * q : 4 * q + 4])
            for i in range(4):
                w16sb[("n", hc, 4 * q + i)] = t[:, i * NC_CHUNK : (i + 1) * NC_CHUNK]

        def wn_pair(h):
            # 256KB: the very first weights the PE touches — finer arrival
            t = wpool.tile([P, 2 * NC_CHUNK], fp16, tag=f"w16n{hc}p{h}", name="wn16sb")
            wn_eng.dma_start(t[:], wn16_d[:, hc, 2 * h : 2 * h + 2])
            for i in range(2):
                w16sb[("n", hc, 2 * h + i)] = t[:, i * NC_CHUNK : (i + 1) * NC_CHUNK]

        def w8(g, kp):
            # NOT quad-packed: a DoubleRow rhs needs a 3D [P,2,NC] AP, and
            # any 3D moving AP carved out of a larger tile (indexed or
            # rearranged) runs the matmul 20-40% slower. Standalone tiles
            # keep the PE at full rate; only flat 2D slices are safe.
            t = wpool.tile([P, 2, NC_CHUNK], fp8, tag=f"w8{g}{hc}k{kp}", name="w8sb")
            nc.sync.dma_start(t[:], w8_d[g][:, hc, kp])
            w8sb[(g, hc, kp)] = t

        def w16(g, ko):
            t = wpool.tile([P, NC_CHUNK], fp16, tag=f"w16{g}{hc}k{ko}", name="w16sb")
            nc.sync.dma_start(t[:], w16_d[g][:, hc, ko])
            w16sb[(g, hc, ko)] = t

        if hc == 0:
            groups = [
                [lambda h=h: wn_pair(h) for h in range(2)]
                + [lambda q=q: wn_quad(q) for q in range(1, KO // 4)]
            ]
        else:
            groups = [[lambda q=q: wn_quad(q) for q in range(KO // 4)]]
        for g, mode in (("r", r_mode), ("z", z_mode)):
            grp = []
            if mode != "fp16":
                for kp in range(KP // 2 if mode == "mixed" else KP):
                    grp.append(lambda g=g, kp=kp: w8(g, kp))
            if mode != "fp8":
                for ko in range(KO // 2 if mode == "mixed" else 0, KO):
                    grp.append(lambda g=g, ko=ko: w16(g, ko))
            groups.append(grp)
        return groups

    with tile.TileContext(nc) as tc:
        with (
            tc.tile_pool(name="wpool", bufs=1) as wpool,
            tc.tile_pool(name="lpool", bufs=1) as lpool,
            tc.tile_pool(name="hpool", bufs=6) as hpool,
            tc.tile_pool(name="epool", bufs=2 if with_bias else 3) as epool,
            tc.tile_pool(name="psum", bufs=2, space="PSUM") as psum,
        ):
            # ---- head: first m-tile's lhsT, then pass-0 weights. All
            # bulk DMAs drip just-in-time through the loop: a clump of big
            # dma_starts fills the DGE ring and blocks the issuing engine.
            lt16 = {}
            lt8 = {}
            hts = {}

            KH = KO // 2

            def issue_lt(m):
                if m == 0:
                    # two half-tiles: the first matmul only waits on 256KB
                    a = lpool.tile([P, KH, P], fp16, tag="lt16_0a", name="lt16sb")
                    nc.sync.dma_start(a[:], lt16_d[0][:, :KH, :])
                    b = lpool.tile([P, KH, P], fp16, tag="lt16_0b", name="lt16sb")
                    nc.sync.dma_start(b[:], lt16_d[0][:, KH:, :])
                    lt16[0] = (a, b)
                    if any_fp8:
                        a8 = lpool.tile([P, KH, P], fp8, tag="lt8_0a", name="lt8sb")
                        nc.sync.dma_start(a8[:], lt8_d[0][:, :KH, :])
                        b8 = lpool.tile([P, KH, P], fp8, tag="lt8_0b", name="lt8sb")
                        nc.sync.dma_start(b8[:], lt8_d[0][:, KH:, :])
                        lt8[0] = (a8, b8)
                    return
                lt16[m] = lpool.tile([P, KO, P], fp16, tag=f"lt16_{m}", name="lt16sb")
                nc.sync.dma_start(lt16[m][:], lt16_d[m])
                if any_fp8:
                    lt8[m] = lpool.tile([P, KO, P], fp8, tag=f"lt8_{m}", name="lt8sb")
                    nc.sync.dma_start(lt8[m][:], lt8_d[m])

            def lt16_sl(m, ko):
                if m == 0:
                    return lt16[0][ko // KH][:, ko % KH, :]
                return lt16[m][:, ko, :]

            def lt8_sl(m, kp):
                # DR lhsT slice [P, 2, P]; ko pair 2kp..2kp+1 never straddles
                # the half boundary (KH=8 even, pairs aligned)
                if m == 0:
                    h = (2 * kp) // KH
                    o = (2 * kp) % KH
                    return lt8[0][h][:, o : o + 2, :]
                return lt8[m][:, 2 * kp : 2 * kp + 2, :]

            def issue_ht(hc, m):
                t = hpool.tile([P, NC_CHUNK], fp16, tag="hnat", name="hsb")
                cs = slice(hc * NC_CHUNK, (hc + 1) * NC_CHUNK)
                nc.sync.dma_start(t[:], h16_d[m * P : (m + 1) * P, cs])
                hts[(hc, m)] = t

            w8sb = {}
            w16sb = {}
            issue_lt(0)
            issue_lt(1)
            wn_g, wr_g, wz_g = weight_issuers(0, wpool, w8sb, w16sb)
            for th in wn_g:  # scalar queue, in parallel with sync below
                th()
            for th in wr_g:
                th()
            issue_ht(0, 0)
            issue_ht(0, 1)
            for th in wz_g:
                th()
            issue_lt(2)
            deferred = [th for grp in weight_issuers(1, wpool, w8sb, w16sb) for th in grp]

            bias_sb = None
            if with_bias:
                bias_sb = wpool.tile([P, 4, H], f32, tag="bias_sb")
                nc.scalar.dma_start(bias_sb[:], bias_d[:])

            # ---- two passes over m-tiles ----
            n_def = len(deferred)
            for hc in range(HC_N):
                cs = slice(hc * NC_CHUNK, (hc + 1) * NC_CHUNK)
                for m in range(n_m):
                    m0 = m * P
                    it = hc * n_m + m
                    # drip: next lhsT tiles, h chunks, and pass-1 weights
                    # (the latter only after the feed-critical first iters)
                    if hc == 0 and m + 3 < n_m:
                        issue_lt(m + 3)
                    la = it + 2
                    if la < HC_N * n_m:
                        issue_ht(la // n_m, la % n_m)
                    if hc == 0 and m >= 3:
                        for _ in range((n_def + n_m - 4) // (n_m - 3)):
                            if deferred:
                                deferred.pop(0)()
                    ht = hts.pop((hc, m))
                    pr = psum.tile([P, NC_CHUNK], f32, tag="pr")
                    pz = psum.tile([P, NC_CHUNK], f32, tag="pz")
                    pxn = psum.tile([P, NC_CHUNK], f32, tag="pxn")
                    phn = psum.tile([P, NC_CHUNK], f32, tag="phn")
                    def emit_n():
                        # xn over ko<KO/2, hn over ko>=KO/2 (fp16)
                        for ko in range(KO):
                            if ko < KO // 2:
                                nc.tensor.matmul(
                                    pxn[:],
                                    lt16_sl(m, ko),
                                    w16sb[("n", hc, ko)],
                                    start=(ko == 0),
                                    stop=(ko == KO // 2 - 1),
                                )
                            else:
                                nc.tensor.matmul(
                                    phn[:],
                                    lt16_sl(m, ko),
                                    w16sb[("n", hc, ko)],
                                    start=(ko == KO // 2),
                                    stop=(ko == KO - 1),
                                )

                    def emit_rz(g, mode, pt):
                        if mode == "fp8":
                            for kp in range(KP):
                                nc.tensor.matmul(
                                    pt[:],
                                    lt8_sl(m, kp),
                                    w8sb[(g, hc, kp)],
                                    start=(kp == 0),
                                    stop=(kp == KP - 1),
                                    perf_mode=DR,
                                )
                        elif mode == "mixed":
                            for kp in range(KP // 2):
                                nc.tensor.matmul(
                                    pt[:],
                                    lt8_sl(m, kp),
                                    w8sb[(g, hc, kp)],
                                    start=(kp == 0),
                                    stop=False,
                                    perf_mode=DR,
                                )
                            for ko in range(KO // 2, KO):
                                nc.tensor.matmul(
                                    pt[:],
                                    lt16_sl(m, ko),
                                    w16sb[(g, hc, ko)],
                                    start=False,
                                    stop=(ko == KO - 1),
                                )
                        else:
                            for ko in range(KO):
                                nc.tensor.matmul(
                                    pt[:],
                                    lt16_sl(m, ko),
                                    w16sb[(g, hc, ko)],
                                    start=(ko == 0),
                                    stop=(ko == KO - 1),
                                )

                    # n first normally (its weights arrive first in the
                    # head); r first on the final iteration so the tanh
                    # chain overlaps the z matmuls and only z's sigmoid +
                    # the blend trail the last matmul.
                    if it == HC_N * n_m - 1:
                        emit_rz("r", r_mode, pr)
                        emit_n()
                        emit_rz("z", z_mode, pz)
                    else:
                        emit_n()
                        emit_rz("r", r_mode, pr)
                        emit_rz("z", z_mode, pz)

                    sr = epool.tile([P, NC_CHUNK], fp16, tag="sr")
                    sz = epool.tile([P, NC_CHUNK], fp16, tag="sz")
                    sn = epool.tile([P, NC_CHUNK], fp16, tag="sn")
                    tt = epool.tile([P, NC_CHUNK], f32, tag="tt")
                    ob = epool.tile([P, NC_CHUNK], fp16, tag="ob")
                    if with_bias:
                        nc.scalar.mul(tt[:], pr[:], gate_scale["r"])
                        nc.vector.tensor_add(tt[:], tt[:], bias_sb[:, 0, cs])
                        nc.scalar.activation(sr[:], tt[:], Sigmoid)
                        nc.scalar.mul(tt[:], pz[:], gate_scale["z"])
                        nc.vector.tensor_add(tt[:], tt[:], bias_sb[:, 1, cs])
                        nc.scalar.activation(sz[:], tt[:], Sigmoid)
                        nc.vector.tensor_add(tt[:], phn[:], bias_sb[:, 3, cs])
                        nc.vector.tensor_mul(tt[:], sr[:], tt[:])
                        nc.vector.tensor_add(tt[:], tt[:], pxn[:])
                        nc.vector.tensor_add(tt[:], tt[:], bias_sb[:, 2, cs])
                        nc.scalar.activation(sn[:], tt[:], Tanh)
                    else:
                        # issue order matters: every op before sz's sigmoid
                        # only needs pr/pxn/phn, so it runs during the z
                        # matmuls; after the last matmul only sz + 3 vector
                        # ops remain.
                        nc.scalar.activation(sr[:], pr[:], Sigmoid, scale=gate_scale["r"])
                        nc.vector.tensor_mul(tt[:], sr[:], phn[:])
                        nc.vector.tensor_add(tt[:], tt[:], pxn[:])
                        nc.scalar.activation(sn[:], tt[:], Tanh)
                        nc.scalar.activation(sz[:], pz[:], Sigmoid, scale=gate_scale["z"])
                    nc.vector.tensor_sub(tt[:], ht[:], sn[:])
                    nc.vector.tensor_mul(tt[:], tt[:], sz[:])
                    nc.vector.tensor_add(ob[:], sn[:], tt[:])
                    nc.scalar.dma_start(out_d[m0 : m0 + P, cs], ob[:])
    nc.finalize()
    return nc


_PROGRAM_CACHE: dict = {}


def get_program(b_core: int, with_bias: bool, r_mode: str = R_MODE, z_mode: str = Z_MODE) -> bass.Bass:
    key = (b_core, with_bias, r_mode, z_mode)
    if key not in _PROGRAM_CACHE:
        _PROGRAM_CACHE[key] = build_gru_program(b_core, with_bias, r_mode, z_mode)
    return _PROGRAM_CACHE[key]


def _to_e4m3(a: np.ndarray, scale: float) -> np.ndarray:
    # this e4m3 variant saturates at 240 and has inf — clip to stay finite
    return np.ascontiguousarray(
        np.clip(a * scale, -240.0, 240.0).astype(ml_dtypes.float8_e4m3)
    )


def _w_fp8_layout(w: np.ndarray) -> np.ndarray:
    """[K, H] f32 -> [P, HC_N, KP, 2, NC_CHUNK] e4m3 (scaled)."""
    a = _to_e4m3(w, W_SCALE)  # [K, H]
    a = a.reshape(KP, 2, P, HC_N, NC_CHUNK)  # k = ((kp*2+j)*128+p)
    return np.ascontiguousarray(a.transpose(2, 3, 0, 1, 4))


def _w_fp16_layout(w: np.ndarray, scale: float = 1.0) -> np.ndarray:
    """[K, H] f32 -> [P, HC_N, KO, NC_CHUNK] f16."""
    a = (w * scale).astype(np.float16).reshape(KO, P, HC_N, NC_CHUNK)
    return np.ascontiguousarray(a.transpose(1, 2, 0, 3))


def prepare_in_maps(h, x, W_ir, W_iz, W_in, b_ir, b_iz, b_in, W_hr, W_hz, W_hn, b_hn,
                    r_mode: str = R_MODE, z_mode: str = Z_MODE):
    """Host-side shard + layout prep. Returns (in_maps, with_bias, b_core)."""
    h = np.ascontiguousarray(np.asarray(h, dtype=np.float32))
    x = np.ascontiguousarray(np.asarray(x, dtype=np.float32))
    b_full = x.shape[0]
    assert b_full % N_CORES == 0
    b_core = b_full // N_CORES
    n_m = b_core // P
    any_fp8 = r_mode != "fp16" or z_mode != "fp16"

    wr_ = np.concatenate([W_ir, W_hr], axis=0).astype(np.float32)
    wz_ = np.concatenate([W_iz, W_hz], axis=0).astype(np.float32)
    wn_ = np.concatenate([W_in, W_hn], axis=0).astype(np.float32)

    # A 'mixed' gate accumulates its fp8 x-half (scaled by ACT_SCALE*W_SCALE)
    # and its fp16 h-half into one psum, so the fp16 half carries the same
    # scale; the sigmoid's scale argument descales the whole sum.
    shared = {"wn16": _w_fp16_layout(wn_)}
    if r_mode != "fp16":
        shared["wr8"] = _w_fp8_layout(wr_)
    if r_mode != "fp8":
        shared["wr16"] = _w_fp16_layout(wr_, ACT_SCALE * W_SCALE if r_mode == "mixed" else 1.0)
    if z_mode != "fp16":
        shared["wz8"] = _w_fp8_layout(wz_)
    if z_mode != "fp8":
        shared["wz16"] = _w_fp16_layout(wz_, ACT_SCALE * W_SCALE if z_mode == "mixed" else 1.0)

    br = np.asarray(b_ir, np.float32)
    bz = np.asarray(b_iz, np.float32)
    bn = np.asarray(b_in, np.float32)
    bhn = np.asarray(b_hn, np.float32)
    biases = np.stack([br, bz, bn, bhn]).astype(np.float32)
    with_bias = bool(np.any(biases != 0.0))
    if with_bias:
        shared["bias_rep"] = np.ascontiguousarray(
            np.broadcast_to(biases[None], (P, 4, H))
        )

    in_maps = []
    for c in range(N_CORES):
        sl = slice(c * b_core, (c + 1) * b_core)
        xc = x[sl]
        hc = h[sl]
        lhsT_full = np.empty((K, b_core), np.float32)
        lhsT_full[:F] = xc.T
        lhsT_full[F:] = hc.T
        # [K, b_core] -> [n_m, P, KO, P]; k = ko*128+p, b = mt*128+m
        lt16 = np.ascontiguousarray(
            lhsT_full.astype(np.float16)
            .reshape(KO, P, n_m, P)
            .transpose(2, 1, 0, 3)
        )
        m = dict(shared)
        m["lt16"] = lt16
        m["h16"] = np.ascontiguousarray(hc.astype(np.float16))
        if any_fp8:
            m["lt8"] = np.ascontiguousarray(
                _to_e4m3(lhsT_full, ACT_SCALE)
                .reshape(KO, P, n_m, P)
                .transpose(2, 1, 0, 3)
            )
        in_maps.append(m)
    return in_maps, with_bias, b_core


def kernel(h, x, W_ir, W_iz, W_in, b_ir, b_iz, b_in, W_hr, W_hz, W_hn, b_hn):
    in_maps, with_bias, b_core = prepare_in_maps(
        h, x, W_ir, W_iz, W_in, b_ir, b_iz, b_in, W_hr, W_hz, W_hn, b_hn
    )
    nc = get_program(b_core, with_bias)
    res = run_bass_kernel_spmd(nc, in_maps, list(range(N_CORES)))
    new_h = np.concatenate(
        [res.results[c]["out"] for c in range(N_CORES)], axis=0
    ).astype(np.float32)
    return (new_h, new_h)


# revision 29
# speedup vs baseline: 1.0153x; 1.0153x over previous
"""Trainium2 Bass kernel for a fused GRUCell step.

Math (reference):
    xi = x @ [W_ir W_iz W_in] + [b_ir b_iz b_in]
    hh = h @ [W_hr W_hz W_hn]
    r = sigmoid(xr + hr); z = sigmoid(xz + hz)
    n = tanh(xn + r * (hn + b_hn))
    new_h = (1 - z) * n + z * h

Strategy: pure data-parallel over the batch dim (B=16384 -> 8 cores x 2048).
Weights replicated. Per core, one K-concatenated GEMM family with K = F + H
= 2048: stationary = batch block of xh^T, moving = per-gate weights.

Precision plan (the kernel is tensor-bound: fp16 PE floor is ~328us/core):
  - r and z gates run in fp8 e4m3 DoubleRow mode (0.5 cycles/row) — their
    quantization error is strongly attenuated through the sigmoid and the
    r*hn / z*(h-n) paths (measured ~1.9e-2 rel err on the full GRU).
  - n gate stays fp16 (it dominates output error; fp8 there fails 2e-2).
  - fp8 operands carry scales (acts x16, weights x512) to stay in e4m3's
    normal range; the descale folds into the sigmoid's scale argument.
  - h for the final blend and the output are fp16 (adds ~2e-4).

DMA plan: two passes over the m-tiles (output columns 0..511, then
512..1023) so the first pass needs only half the weights and the PE starts
~10us in; lhsT stays resident in SBUF across both passes. Bulk loads issue
on the sync (SP) DGE queue, dripped just-in-time through the loop — a
clump of large dma_starts fills the DGE ring and blocks the issuing
engine, which would stall compute if issued from scalar (measured: a 37us
PE stall). During the head only, pass-0's n-gate weights and first h
chunks ride the still-idle scalar queue so both sequencers generate
descriptors in parallel; mid-run the scalar queue only writes outputs.

Measured: 246us (baseline fp16 kernel: 401us); fp16 PE floor 328us, this
mix's floor 218us busy + ~8us preamble + ~7us tail.
"""

import os
import sys

import numpy as np

sys.path.insert(0, "/opt/trn_rl_repo")
os.environ.setdefault("MYCRO_LOCAL_CACHE", "1")

import ml_dtypes  # noqa: E402

import concourse.bass as bass  # noqa: E402
import concourse.mybir as mybir  # noqa: E402
import concourse.tile as tile  # noqa: E402
from concourse import bacc  # noqa: E402
from concourse.bass_utils import run_bass_kernel_spmd  # noqa: E402

N_CORES = 8
F = 1024  # input feature dim
H = 1024  # hidden dim
K = F + H  # GEMM contraction dim (x features then h features)
P = 128
KO = K // P  # 16 k-chunks of 128
KP = KO // 2  # 8 k-chunk pairs (DoubleRow processes 2 at a time)
NC_CHUNK = 512  # H columns per PSUM bank / matmul
HC_N = H // NC_CHUNK

ACT_SCALE = 16.0  # x,h ~ N(0,1) -> fp8 values ~N(0,16), well inside e4m3
W_SCALE = 512.0  # W ~ N(0,1/1024) -> fp8 values ~N(0,16)
INV_SCALE = 1.0 / (ACT_SCALE * W_SCALE)

# Per-gate precision for r and z: 'fp8' (both K-halves e4m3 DoubleRow),
# 'mixed' (x-half fp8, h-half fp16), or 'fp16'. n is always fp16.
R_MODE = "fp8"
Z_MODE = "fp8"


def build_gru_program(b_core: int, with_bias: bool, r_mode: str, z_mode: str) -> bass.Bass:
    """One SPMD program; every core runs it on its own batch shard.

    Two passes over the 16 m-tiles: pass hc=0 computes output columns
    0..511 (needs only half the weights, so the PE starts almost
    immediately and is never starved by the weight stream), pass hc=1 the
    rest. lhsT tiles stay resident in SBUF across both passes.
    """
    fp8 = mybir.dt.float8e4
    fp16 = mybir.dt.float16
    f32 = mybir.dt.float32
    n_m = b_core // P  # m-tiles of 128 batch rows
    assert b_core % P == 0
    any_fp8 = r_mode != "fp16" or z_mode != "fp16"
    DR = mybir.MatmulPerfMode.DoubleRow

    nc = bacc.Bacc()
    # Host-pretransposed layouts: partition dim first, contiguous per-line.
    lt16_d = nc.declare_dram_parameter("lt16", [n_m, P, KO, P], fp16, isOutput=False)
    if any_fp8:
        lt8_d = nc.declare_dram_parameter("lt8", [n_m, P, KO, P], fp8, isOutput=False)
    # r/z weights: [p, hc, kp, j, n] (j = the 2 k-subtiles of a DoubleRow pair)
    w8_d = {}
    w16_d = {}
    for g, mode in (("r", r_mode), ("z", z_mode)):
        if mode != "fp16":
            w8_d[g] = nc.declare_dram_parameter(
                f"w{g}8", [P, HC_N, KP, 2, NC_CHUNK], fp8, isOutput=False
            )
        if mode != "fp8":
            w16_d[g] = nc.declare_dram_parameter(
                f"w{g}16", [P, HC_N, KO, NC_CHUNK], fp16, isOutput=False
            )
    wn16_d = nc.declare_dram_parameter("wn16", [P, HC_N, KO, NC_CHUNK], fp16, isOutput=False)
    h16_d = nc.declare_dram_parameter("h16", [b_core, H], fp16, isOutput=False)
    if with_bias:
        # host-replicated across partitions; rows: b_r, b_z, b_in, b_hn
        bias_d = nc.declare_dram_parameter("bias_rep", [P, 4, H], f32, isOutput=False)
    out_d = nc.declare_dram_parameter("out", [b_core, H], fp16, isOutput=True)

    Sigmoid = mybir.ActivationFunctionType.Sigmoid
    Tanh = mybir.ActivationFunctionType.Tanh

    # Scale of each gate's psum relative to the true pre-activation.
    gate_scale = {
        "r": INV_SCALE if r_mode != "fp16" else 1.0,
        "z": INV_SCALE if z_mode != "fp16" else 1.0,
    }

    def weight_issuers(hc, wpool, w8sb, w16sb):
        """Thunks that each DMA one weight tile of pass `hc` (on sync: the
        SP queue has no compute, so a deep DGE ring is harmless). One tile
        per matmul operand: feeding matmuls from slices of a larger tile
        measured 20% slower per matmul (259ns vs 216ns — the sliced moving
        AP appears to defeat the LDWEIGHTS/stream overlap).

        Descriptor generation costs ~600ns per dma_start and paces the
        head: pass-0's n-gate weights go on the scalar queue (idle until
        the first sigmoid ~20us in) so both sequencers generate
        descriptors in parallel during the head burst."""
        wn_eng = nc.scalar if hc == 0 else nc.sync

        def wn_quad(q):
            # 512KB flat tile, one dma, 4 matmul operands as last-dim
            # slices. Unlike middle-dim-indexed slices (the 20% regression),
            # a contiguous 2D slice feeds the PE at full rate.
            t = wpool.tile([P, 4 * NC_CHUNK], fp16, tag=f"w16n{hc}q{q}", name="wn16sb")
            wn_eng.dma_start(t[:], wn16_d[:, hc, 4 * q : 4 * q + 4])
            for i in range(4):
                w16sb[("n", hc, 4 * q + i)] = t[:, i * NC_CHUNK : (i + 1) * NC_CHUNK]

        def w8(g, kp):
            # NOT quad-packed: a DoubleRow rhs needs a 3D [P,2,NC] AP, and
            # any 3D moving AP carved out of a larger tile (indexed or
            # rearranged) runs the matmul 20-40% slower. Standalone tiles
            # keep the PE at full rate; only flat 2D slices are safe.
            t = wpool.tile([P, 2, NC_CHUNK], fp8, tag=f"w8{g}{hc}k{kp}", name="w8sb")
            nc.sync.dma_start(t[:], w8_d[g][:, hc, kp])
            w8sb[(g, hc, kp)] = t

        def w16(g, ko):
            t = wpool.tile([P, NC_CHUNK], fp16, tag=f"w16{g}{hc}k{ko}", name="w16sb")
            nc.sync.dma_start(t[:], w16_d[g][:, hc, ko])
            w16sb[(g, hc, ko)] = t

        groups = [[lambda q=q: wn_quad(q) for q in range(KO // 4)]]
        for g, mode in (("r", r_mode), ("z", z_mode)):
            grp = []
            if mode != "fp16":
                for kp in range(KP // 2 if mode == "mixed" else KP):
                    grp.append(lambda g=g, kp=kp: w8(g, kp))
            if mode != "fp8":
                for ko in range(KO // 2 if mode == "mixed" else 0, KO):
                    grp.append(lambda g=g, ko=ko: w16(g, ko))
            groups.append(grp)
        return groups

    with tile.TileContext(nc) as tc:
        with (
            tc.tile_pool(name="wpool", bufs=1) as wpool,
            tc.tile_pool(name="lpool", bufs=1) as lpool,
            tc.tile_pool(name="hpool", bufs=6) as hpool,
            tc.tile_pool(name="epool", bufs=2 if with_bias else 3) as epool,
            tc.tile_pool(name="psum", bufs=2, space="PSUM") as psum,
        ):
            # ---- head: first m-tile's lhsT, then pass-0 weights. All
            # bulk DMAs drip just-in-time through the loop: a clump of big
            # dma_starts fills the DGE ring and blocks the issuing engine.
            lt16 = {}
            lt8 = {}
            hts = {}

            def issue_lt(m):
                lt16[m] = lpool.tile([P, KO, P], fp16, tag=f"lt16_{m}", name="lt16sb")
                nc.sync.dma_start(lt16[m][:], lt16_d[m])
                if any_fp8:
                    lt8[m] = lpool.tile([P, KO, P], fp8, tag=f"lt8_{m}", name="lt8sb")
                    nc.sync.dma_start(lt8[m][:], lt8_d[m])

            def issue_ht(hc, m):
                t = hpool.tile([P, NC_CHUNK], fp16, tag="hnat", name="hsb")
                cs = slice(hc * NC_CHUNK, (hc + 1) * NC_CHUNK)
                nc.sync.dma_start(t[:], h16_d[m * P : (m + 1) * P, cs])
                hts[(hc, m)] = t

            w8sb = {}
            w16sb = {}
            issue_lt(0)
            issue_lt(1)
            wn_g, wr_g, wz_g = weight_issuers(0, wpool, w8sb, w16sb)
            for th in wn_g:  # scalar queue, in parallel with sync below
                th()
            for th in wr_g:
                th()
            issue_ht(0, 0)
            issue_ht(0, 1)
            for th in wz_g:
                th()
            issue_lt(2)
            deferred = [th for grp in weight_issuers(1, wpool, w8sb, w16sb) for th in grp]

            bias_sb = None
            if with_bias:
                bias_sb = wpool.tile([P, 4, H], f32, tag="bias_sb")
                nc.scalar.dma_start(bias_sb[:], bias_d[:])

            # ---- two passes over m-tiles ----
            n_def = len(deferred)
            for hc in range(HC_N):
                cs = slice(hc * NC_CHUNK, (hc + 1) * NC_CHUNK)
                for m in range(n_m):
                    m0 = m * P
                    it = hc * n_m + m
                    # drip: next lhsT tiles, h chunks, and pass-1 weights
                    # (the latter only after the feed-critical first iters)
                    if hc == 0 and m + 3 < n_m:
                        issue_lt(m + 3)
                    la = it + 2
                    if la < HC_N * n_m:
                        issue_ht(la // n_m, la % n_m)
                    if hc == 0 and m >= 3:
                        for _ in range((n_def + n_m - 4) // (n_m - 3)):
                            if deferred:
                                deferred.pop(0)()
                    ht = hts.pop((hc, m))
                    pr = psum.tile([P, NC_CHUNK], f32, tag="pr")
                    pz = psum.tile([P, NC_CHUNK], f32, tag="pz")
                    pxn = psum.tile([P, NC_CHUNK], f32, tag="pxn")
                    phn = psum.tile([P, NC_CHUNK], f32, tag="phn")
                    # n gate first: xn over ko<KO/2, hn over ko>=KO/2 (fp16)
                    for ko in range(KO):
                        if ko < KO // 2:
                            nc.tensor.matmul(
                                pxn[:],
                                lt16[m][:, ko, :],
                                w16sb[("n", hc, ko)],
                                start=(ko == 0),
                                stop=(ko == KO // 2 - 1),
                            )
                        else:
                            nc.tensor.matmul(
                                phn[:],
                                lt16[m][:, ko, :],
                                w16sb[("n", hc, ko)],
                                start=(ko == KO // 2),
                                stop=(ko == KO - 1),
                            )
                    # r and z gates
                    for g, mode, pt in (("r", r_mode, pr), ("z", z_mode, pz)):
                        if mode == "fp8":
                            for kp in range(KP):
                                nc.tensor.matmul(
                                    pt[:],
                                    lt8[m][:, 2 * kp : 2 * kp + 2, :],
                                    w8sb[(g, hc, kp)],
                                    start=(kp == 0),
                                    stop=(kp == KP - 1),
                                    perf_mode=DR,
                                )
                        elif mode == "mixed":
                            for kp in range(KP // 2):
                                nc.tensor.matmul(
                                    pt[:],
                                    lt8[m][:, 2 * kp : 2 * kp + 2, :],
                                    w8sb[(g, hc, kp)],
                                    start=(kp == 0),
                                    stop=False,
                                    perf_mode=DR,
                                )
                            for ko in range(KO // 2, KO):
                                nc.tensor.matmul(
                                    pt[:],
                                    lt16[m][:, ko, :],
                                    w16sb[(g, hc, ko)],
                                    start=False,
                                    stop=(ko == KO - 1),
                                )
                        else:
                            for ko in range(KO):
                                nc.tensor.matmul(
                                    pt[:],
                                    lt16[m][:, ko, :],
                                    w16sb[(g, hc, ko)],
                                    start=(ko == 0),
                                    stop=(ko == KO - 1),
                                )

                    sr = epool.tile([P, NC_CHUNK], fp16, tag="sr")
                    sz = epool.tile([P, NC_CHUNK], fp16, tag="sz")
                    sn = epool.tile([P, NC_CHUNK], fp16, tag="sn")
                    tt = epool.tile([P, NC_CHUNK], f32, tag="tt")
                    ob = epool.tile([P, NC_CHUNK], fp16, tag="ob")
                    if with_bias:
                        nc.scalar.mul(tt[:], pr[:], gate_scale["r"])
                        nc.vector.tensor_add(tt[:], tt[:], bias_sb[:, 0, cs])
                        nc.scalar.activation(sr[:], tt[:], Sigmoid)
                        nc.scalar.mul(tt[:], pz[:], gate_scale["z"])
                        nc.vector.tensor_add(tt[:], tt[:], bias_sb[:, 1, cs])
                        nc.scalar.activation(sz[:], tt[:], Sigmoid)
                        nc.vector.tensor_add(tt[:], phn[:], bias_sb[:, 3, cs])
                        nc.vector.tensor_mul(tt[:], sr[:], tt[:])
                        nc.vector.tensor_add(tt[:], tt[:], pxn[:])
                        nc.vector.tensor_add(tt[:], tt[:], bias_sb[:, 2, cs])
                        nc.scalar.activation(sn[:], tt[:], Tanh)
                    else:
                        # issue order matters: every op before sz's sigmoid
                        # only needs pr/pxn/phn, so it runs during the z
                        # matmuls; after the last matmul only sz + 3 vector
                        # ops remain.
                        nc.scalar.activation(sr[:], pr[:], Sigmoid, scale=gate_scale["r"])
                        nc.vector.tensor_mul(tt[:], sr[:], phn[:])
                        nc.vector.tensor_add(tt[:], tt[:], pxn[:])
                        nc.scalar.activation(sn[:], tt[:], Tanh)
                        nc.scalar.activation(sz[:], pz[:], Sigmoid, scale=gate_scale["z"])
                    nc.vector.tensor_sub(tt[:], ht[:], sn[:])
                    nc.vector.tensor_mul(tt[:], tt[:], sz[:])
                    nc.vector.tensor_add(ob[:], sn[:], tt[:])
                    nc.scalar.dma_start(out_d[m0 : m0 + P, cs], ob[:])
    nc.finalize()
    return nc


_PROGRAM_CACHE: dict = {}


def get_program(b_core: int, with_bias: bool, r_mode: str = R_MODE, z_mode: str = Z_MODE) -> bass.Bass:
    key = (b_core, with_bias, r_mode, z_mode)
    if key not in _PROGRAM_CACHE:
        _PROGRAM_CACHE[key] = build_gru_program(b_core, with_bias, r_mode, z_mode)
    return _PROGRAM_CACHE[key]


def _to_e4m3(a: np.ndarray, scale: float) -> np.ndarray:
    # this e4m3 variant saturates at 240 and has inf — clip to stay finite
    return np.ascontiguousarray(
        np.clip(a * scale, -240.0, 240.0).astype(ml_dtypes.float8_e4m3)
    )


def _w_fp8_layout(w: np.ndarray) -> np.ndarray:
    """[K, H] f32 -> [P, HC_N, KP, 2, NC_CHUNK] e4m3 (scaled)."""
    a = _to_e4m3(w, W_SCALE)  # [K, H]
    a = a.reshape(KP, 2, P, HC_N, NC_CHUNK)  # k = ((kp*2+j)*128+p)
    return np.ascontiguousarray(a.transpose(2, 3, 0, 1, 4))


def _w_fp16_layout(w: np.ndarray, scale: float = 1.0) -> np.ndarray:
    """[K, H] f32 -> [P, HC_N, KO, NC_CHUNK] f16."""
    a = (w * scale).astype(np.float16).reshape(KO, P, HC_N, NC_CHUNK)
    return np.ascontiguousarray(a.transpose(1, 2, 0, 3))


def prepare_in_maps(h, x, W_ir, W_iz, W_in, b_ir, b_iz, b_in, W_hr, W_hz, W_hn, b_hn,
                    r_mode: str = R_MODE, z_mode: str = Z_MODE):
    """Host-side shard + layout prep. Returns (in_maps, with_bias, b_core)."""
    h = np.ascontiguousarray(np.asarray(h, dtype=np.float32))
    x = np.ascontiguousarray(np.asarray(x, dtype=np.float32))
    b_full = x.shape[0]
    assert b_full % N_CORES == 0
    b_core = b_full // N_CORES
    n_m = b_core // P
    any_fp8 = r_mode != "fp16" or z_mode != "fp16"

    wr_ = np.concatenate([W_ir, W_hr], axis=0).astype(np.float32)
    wz_ = np.concatenate([W_iz, W_hz], axis=0).astype(np.float32)
    wn_ = np.concatenate([W_in, W_hn], axis=0).astype(np.float32)

    # A 'mixed' gate accumulates its fp8 x-half (scaled by ACT_SCALE*W_SCALE)
    # and its fp16 h-half into one psum, so the fp16 half carries the same
    # scale; the sigmoid's scale argument descales the whole sum.
    shared = {"wn16": _w_fp16_layout(wn_)}
    if r_mode != "fp16":
        shared["wr8"] = _w_fp8_layout(wr_)
    if r_mode != "fp8":
        shared["wr16"] = _w_fp16_layout(wr_, ACT_SCALE * W_SCALE if r_mode == "mixed" else 1.0)
    if z_mode != "fp16":
        shared["wz8"] = _w_fp8_layout(wz_)
    if z_mode != "fp8":
        shared["wz16"] = _w_fp16_layout(wz_, ACT_SCALE * W_SCALE if z_mode == "mixed" else 1.0)

    br = np.asarray(b_ir, np.float32)
    bz = np.asarray(b_iz, np.float32)
    bn = np.asarray(b_in, np.float32)
    bhn = np.asarray(b_hn, np.float32)
    biases = np.stack([br, bz, bn, bhn]).astype(np.float32)
    with_bias = bool(np.any(biases != 0.0))
    if with_bias:
        shared["bias_rep"] = np.ascontiguousarray(
            np.broadcast_to(biases[None], (P, 4, H))
        )

    in_maps = []
    for c in range(N_CORES):
        sl = slice(c * b_core, (c + 1) * b_core)
        xc = x[sl]
        hc = h[sl]
        lhsT_full = np.empty((K, b_core), np.float32)
        lhsT_full[:F] = xc.T
        lhsT_full[F:] = hc.T
        # [K, b_core] -> [n_m, P, KO, P]; k = ko*128+p, b = mt*128+m
        lt16 = np.ascontiguousarray(
            lhsT_full.astype(np.float16)
            .reshape(KO, P, n_m, P)
            .transpose(2, 1, 0, 3)
        )
        m = dict(shared)
        m["lt16"] = lt16
        m["h16"] = np.ascontiguousarray(hc.astype(np.float16))
        if any_fp8:
            m["lt8"] = np.ascontiguousarray(
                _to_e4m3(lhsT_full, ACT_SCALE)
                .reshape(KO, P, n_m, P)
                .transpose(2, 1, 0, 3)
            )
        in_maps.append(m)
    return in_maps, with_bias, b_core


def kernel(h, x, W_ir, W_iz, W_in, b_ir, b_iz, b_in, W_hr, W_hz, W_hn, b_hn):
    in_maps, with_bias, b_core = prepare_in_maps(
        h, x, W_ir, W_iz, W_in, b_ir, b_iz, b_in, W_hr, W_hz, W_hn, b_hn
    )
    nc = get_program(b_core, with_bias)
    res = run_bass_kernel_spmd(nc, in_maps, list(range(N_CORES)))
    new_h = np.concatenate(
        [res.results[c]["out"] for c in range(N_CORES)], axis=0
    ).astype(np.float32)
    return (new_h, new_h)
